# revision 2
# baseline (speedup 1.0000x reference)
"""Trainium2 Bass kernel for nn_AttentionAggregator.

Computation (per side, users/items symmetric):
    cu  = concat(gather(review_vecs, adj_r), gather(sec_vecs, adj_s))   # [6000, 1024]
    att = softmax(keys @ keys.T / 8) @ cu                               # [6000, 1024]
    out = relu(att @ W)                                                 # [6000, 1024]

Sharding: 8 cores run the same program (SPMD). Cores 0-3 take the user side
(1500 query rows each), cores 4-7 the item side. Keys, gather sources,
adjacency and weights are replicated; only the query slice differs.

On-device per core:
  - gather cu ONE indirect DMA per chunk of 8 k-tiles (review_vecs and the
    secondary source are concatenated on host into one DRAM tensor, and the
    adjacency indices merged/offset to match, so a chunk's whole [128, 8*16]
    index block drives a single 16k-descriptor gather). This keeps the
    serialized per-call SWDGE overhead (~1us each) off the critical path.
  - scoresT[k,q] = keys @ q.T via PE in float32r (4x the fp32 rate)
  - E = exp(scoresT/8) on ScalarE directly PSUM->SBUF (no max-subtraction
    needed: |scores| <= ~14 in fp32)
  - O = E.T-weighted sum of cu, accumulated on PE in PSUM over each chunk,
    then folded into an SBUF fp32 accumulator by DVE
  - rowsums r = E.T @ ones accumulated in a persistent PSUM bank
  - out = relu(O @ W) * (1/r), with the 1/r per-partition scale fused into
    the final ReLU PSUM->SBUF copy (valid since r > 0)

Column layout of the gathered cu is [review slots 0-7 | sec slots 0-7]
(instead of the reference's interleaved layout); the host permutes W's rows
to match, so results are identical.
"""

import os
import sys

import ml_dtypes
import numpy as np

for _p in ("/opt/trn_rl_repo", "/root/.axon_site/_ro/trn_rl_repo"):
    if os.path.isdir(_p) and _p not in sys.path:
        sys.path.append(_p)

import concourse.bass as bass  # noqa: E402
import concourse.mybir as mybir  # noqa: E402
import concourse.tile as tile  # noqa: E402
from concourse import bacc  # noqa: E402
from concourse.bass_utils import run_bass_kernel_spmd  # noqa: E402
from concourse.masks import make_identity  # noqa: E402

P = 128
D = 64
NK = 6000          # keys per side
NKP = 6144         # padded to 48 full k-tiles
KT = NKP // P      # 48
KT_CALC = 47       # k-tiles that carry real keys (kt 47 is all padding)
QOUT = 1500        # query rows per core (6000 / 4 cores per side)
QP = 1536          # padded to 12 full q-subtiles
NQS = QP // P      # 12
HID = 1024
NR = 30000         # review_vecs rows
NS = 6000          # secondary source rows
NCAT = NR + NS     # concatenated gather source rows
# chunk sizes; small first chunk so the PE pipeline fills early
CHUNK_SIZES = tuple(int(x) for x in os.environ.get("K_CHUNKS", "4,8,8,8,8,8,4").split(","))
assert sum(CHUNK_SIZES) == 48
CHUNK_STARTS = tuple(int(np.cumsum((0,) + CHUNK_SIZES)[i]) for i in range(len(CHUNK_SIZES)))
F32 = mybir.dt.float32
F32R = mybir.dt.float32r
BF16 = mybir.dt.bfloat16
I32 = mybir.dt.int32

AF = mybir.ActivationFunctionType


def _emit_body(nc, tc, ctx_pools, tensors):
    """Emit one full pass of the kernel body inside an open TileContext."""
    from contextlib import ExitStack

    keysT, qvT, adj, src_cat, w, ebias, out = tensors
    const, psum, psum_r = ctx_pools

    # ---- persistent tiles -------------------------------------------------
    identity = const.tile([P, P], F32, tag="identity")
    make_identity(nc, identity[:])
    ones = const.tile([P, 1], BF16, tag="ones")
    nc.gpsimd.memset(ones[:], 1.0)

    vecsT = const.tile([P, NKP], F32R, tag="vecsT")
    nc.any.memzero(vecsT[D:, :])
    nc.sync.dma_start(vecsT[:D, :], keysT[:, :])

    qvT_sb = const.tile([P, QP], F32R, tag="qvT")
    nc.any.memzero(qvT_sb[D:, :])
    nc.sync.dma_start(qvT_sb[:D, :], qvT[:, :])

    adj_sb = const.tile([P, KT * 16], I32, tag="adj")
    nc.sync.dma_start(adj_sb[:], adj[:, :])

    ebias_sb = const.tile([P, 1], F32, tag="ebias")
    nc.sync.dma_start(ebias_sb[:], ebias[:, :])

    # weights for phase B: load early so the DMA overlaps phase A compute
    w_sb = const.tile([P, HID // P, HID], BF16, tag="w")
    nc.sync.dma_start(w_sb[:], w[:, :, :])

    o_acc = const.tile([P, NQS, HID], F32, tag="oacc")
    r_acc = const.tile([P, NQS], F32, tag="racc")
    rinv = const.tile([P, NQS], F32, tag="rinv")

    chunks = [list(range(st, min(st + cs, KT_CALC)))
              for st, cs in zip(CHUNK_STARTS, CHUNK_SIZES)]

    # ---- phase A: attention numerator + rowsums ---------------------------
    with ExitStack() as ctx:
        e_pool = ctx.enter_context(tc.tile_pool(name="e_pool", bufs=2))
        g_pool = ctx.enter_context(tc.tile_pool(name="g_pool", bufs=3))

        for ci, chunk in enumerate(chunks):
            first_chunk = ci == 0
            st = CHUNK_STARTS[ci]
            n = len(chunk)

            # one gather for the whole chunk: 16 indices per (partition, kt)
            # -> 16 rows of 64 elements each = the full 1024-col cu row
            g = g_pool.tile([P, n, HID], BF16, tag="g")
            nc.gpsimd.indirect_dma_start(
                out=g[:, :, :],
                out_offset=None,
                in_=src_cat[:],
                in_offset=bass.IndirectOffsetOnAxis(
                    ap=adj_sb[:, st * 16:(st + n) * 16], axis=0),
            )

            e = e_pool.tile([P, n, QP], BF16, tag="e")
            for t, kt in enumerate(chunk):
                lhsT = vecsT[:, kt * P:(kt + 1) * P]
                for i in range(QP // 512):
                    s_ps = psum.tile([P, 512], F32, tag="ps")
                    nc.tensor.matmul(
                        s_ps[:], lhsT, qvT_sb[:, i * 512:(i + 1) * 512],
                        start=True, stop=True,
                    )
                    # padded key rows (6000..6015) get bias -1e30 so
                    # exp() forces their attention weight to exactly zero
                    bias = ebias_sb[:, 0:1] if kt == KT_CALC - 1 else 0.0
                    nc.scalar.activation(
                        e[:, t, i * 512:(i + 1) * 512], s_ps[:], AF.Exp,
                        bias=bias, scale=0.125,
                    )

            r_ps = psum_r.tile([P, NQS], F32, tag="rps")
            for j in range(NQS):
                p0 = psum.tile([P, 512], F32, tag="ps")
                p1 = psum.tile([P, 512], F32, tag="ps")
                for t, kt in enumerate(chunk):
                    lhsT = e[:, t, j * P:(j + 1) * P]
                    first = t == 0
                    last = t == n - 1
                    nc.tensor.matmul(p0[:], lhsT, g[:, t, 0:512],
                                     start=first, stop=last)
                    nc.tensor.matmul(p1[:], lhsT, g[:, t, 512:1024],
                                     start=first, stop=last)
                    nc.tensor.matmul(r_ps[:, j:j + 1], lhsT, ones[:],
                                     start=first, stop=last)
                if first_chunk:
                    nc.vector.tensor_copy(o_acc[:, j, 0:512], p0[:])
                    nc.vector.tensor_copy(o_acc[:, j, 512:1024], p1[:])
                else:
                    nc.vector.tensor_add(o_acc[:, j, 0:512], o_acc[:, j, 0:512], p0[:])
                    nc.vector.tensor_add(o_acc[:, j, 512:1024], o_acc[:, j, 512:1024], p1[:])
            if first_chunk:
                nc.vector.tensor_copy(r_acc[:], r_ps[:])
            else:
                nc.vector.tensor_add(r_acc[:], r_acc[:], r_ps[:])

    # ---- phase B: normalize (folded), project through W, relu, store ------
    nc.vector.reciprocal(rinv[:], r_acc[:])

    with ExitStack() as ctx:
        ot_pool = ctx.enter_context(tc.tile_pool(name="ot_pool", bufs=10))
        ob_pool = ctx.enter_context(tc.tile_pool(name="ob_pool", bufs=4))

        for j in range(NQS):
            ots = []
            for t in range(HID // P):
                tp = psum.tile([P, 512], F32, tag="ps")
                nc.tensor.transpose(
                    tp[:, 0:P], o_acc[:, j, t * P:(t + 1) * P], identity[:],
                )
                ot = ot_pool.tile([P, P], BF16, tag="ot")
                nc.vector.tensor_copy(ot[:], tp[:, 0:P])
                ots.append(ot)
            for h in range(HID // 512):
                pf = psum.tile([P, 512], F32, tag="ps")
                for t in range(HID // P):
                    nc.tensor.matmul(
                        pf[:], ots[t][:], w_sb[:, t, h * 512:(h + 1) * 512],
                        start=(t == 0), stop=(t == HID // P - 1),
                    )
                ob = ob_pool.tile([P, 512], F32, tag="ob")
                nc.scalar.activation(ob[:], pf[:], AF.Relu, scale=rinv[:, j:j + 1])
                rows = min(P, QOUT - j * P)
                if rows > 0:
                    nc.sync.dma_start(
                        out[j * P:j * P + rows, h * 512:(h + 1) * 512], ob[:rows, :],
                    )


def build_program(repeat: int = 0, scratch: int | None = None):
    """Build + compile the SPMD program. repeat>0 wraps the body in a
    device-side For loop (for timing) and is not used for grading."""
    from contextlib import ExitStack

    kw = {} if scratch is None else dict(dynamic_dma_scratch_size=scratch)
    nc = bacc.Bacc("TRN2", target_bir_lowering=False, debug=False, num_devices=8, **kw)

    keysT = nc.dram_tensor("keysT", [D, NKP], F32R, kind="ExternalInput")
    qvT = nc.dram_tensor("qvT", [D, QP], F32R, kind="ExternalInput")
    adj = nc.dram_tensor("adj", [P, KT * 16], I32, kind="ExternalInput")
    src_cat = nc.dram_tensor("src_cat", [NCAT, D], BF16, kind="ExternalInput")
    w = nc.dram_tensor("w", [P, HID // P, HID], BF16, kind="ExternalInput")
    ebias = nc.dram_tensor("ebias", [P, 1], F32, kind="ExternalInput")
    out = nc.dram_tensor("out", [QOUT, HID], F32, kind="ExternalOutput")

    tensors = (keysT, qvT, adj, src_cat, w, ebias, out)

    with tile.TileContext(nc) as tc, ExitStack() as ctx:
        const = ctx.enter_context(tc.tile_pool(name="const", bufs=1))
        psum = ctx.enter_context(tc.tile_pool(name="psum", bufs=6, space="PSUM"))
        psum_r = ctx.enter_context(tc.tile_pool(name="psum_r", bufs=2, space="PSUM"))
        pools = (const, psum, psum_r)
        for _ in range(max(repeat, 1)):
            _emit_body(nc, tc, pools, tensors)

    nc.compile()
    return nc


def _permute_w(w_full: np.ndarray) -> np.ndarray:
    """Reference cu columns are slot-interleaved [r0 i0 r1 i1 ...]; the kernel
    gathers [r0..r7 | i0..i7]. Permute W rows to match, then pre-tile to
    [128, 8, 1024] for the on-device layout."""
    wr = w_full.reshape(8, 2, D, HID)
    w_perm = np.concatenate(
        [wr[:, 0].reshape(8 * D, HID), wr[:, 1].reshape(8 * D, HID)], axis=0,
    )
    return np.ascontiguousarray(
        w_perm.reshape(HID // P, P, HID).transpose(1, 0, 2),
    )


def _merge_adj(adj_r: np.ndarray, adj_s: np.ndarray) -> np.ndarray:
    """[6000, 8] x2 -> [128, KT*16] int32 into the concatenated source:
    per (partition, k-tile) the 16 indices are [r0..r7, s0+NR..s7+NR].
    Padded rows index row 0 (their attention weight is forced to zero)."""
    a = np.zeros((NKP, 16), dtype=np.int32)
    a[:NK, 0:8] = adj_r
    a[:NK, 8:16] = adj_s + NR
    # [NKP, 16] -> [KT, 128, 16] -> [128, KT, 16] -> [128, KT*16]
    return np.ascontiguousarray(
        a.reshape(KT, P, 16).transpose(1, 0, 2).reshape(P, KT * 16))


def _host_inputs(review_vecs, user_vecs, item_vecs, user_weights, item_weights,
                 user_review_adj, user_item_adj, item_review_adj, item_user_adj):
    review_vecs = np.asarray(review_vecs, dtype=np.float32)
    user_vecs = np.asarray(user_vecs, dtype=np.float32)
    item_vecs = np.asarray(item_vecs, dtype=np.float32)
    review_b = review_vecs.astype(ml_dtypes.bfloat16)

    sides = {}
    for side, keys, adj_r, adj_s, src_s, w_full in (
        ("user", user_vecs, user_review_adj, user_item_adj, item_vecs, user_weights),
        ("item", item_vecs, item_review_adj, item_user_adj, user_vecs, item_weights),
    ):
        keysT = np.zeros((D, NKP), dtype=np.float32)
        keysT[:, :NK] = keys.T
        sides[side] = dict(
            keysT=keysT,
            adj=_merge_adj(np.asarray(adj_r, dtype=np.int32),
                           np.asarray(adj_s, dtype=np.int32)),
            src_cat=np.concatenate(
                [review_b, src_s.astype(ml_dtypes.bfloat16)], axis=0),
            w=_permute_w(np.asarray(w_full, dtype=np.float32)).astype(ml_dtypes.bfloat16),
            keys=keys,
        )

    ebias = np.zeros((P, 1), dtype=np.float32)
    ebias[NK - (KT_CALC - 1) * P:] = -1e30

    in_maps = []
    for c in range(8):
        s = sides["user" if c < 4 else "item"]
        b = c % 4
        qv = s["keys"][b * QOUT:(b + 1) * QOUT]  # [1500, 64]
        qvT = np.empty((D, QP), dtype=np.float32)
        qvT[:, :QOUT] = qv.T
        qvT[:, QOUT:] = qv.T[:, :QP - QOUT]  # pad with real vectors (finite rowsums)
        in_maps.append(dict(
            keysT=s["keysT"], qvT=np.ascontiguousarray(qvT),
            adj=s["adj"], src_cat=s["src_cat"], w=s["w"], ebias=ebias,
        ))
    return in_maps


_NC_CACHE = None


def kernel(**inputs):
    global _NC_CACHE
    if _NC_CACHE is None:
        _NC_CACHE = build_program()
    nc = _NC_CACHE
    in_maps = _host_inputs(**inputs)
    res = run_bass_kernel_spmd(nc, in_maps, core_ids=list(range(8)))
    outs = [res.results[c]["out"] for c in range(8)]
    user_output = np.concatenate(outs[0:4], axis=0)
    item_output = np.concatenate(outs[4:8], axis=0)
    return user_output, item_output


# revision 29
# speedup vs baseline: 1.1390x; 1.1390x over previous
"""Trainium2 Bass kernel for nn_AttentionAggregator.

Computation (per side, users/items symmetric):
    cu  = concat(gather(review_vecs, adj_r), gather(sec_vecs, adj_s))   # [6000, 1024]
    att = softmax(keys @ keys.T / 8) @ cu                               # [6000, 1024]
    out = relu(att @ W)                                                 # [6000, 1024]

Sharding: 8 cores run the same program (SPMD). Cores 0-3 take the user side
(1500 query rows each), cores 4-7 the item side. Keys, gather sources,
adjacency and weights are replicated; only the query slice differs.

On-device per core:
  - neighbor features are gathered with the custom `dma_gather` ucode
    (InstDMAGatherAnt): one call per (pair of k-tiles, source) moves 2048
    rows of 64 fp32 (256B blocks - the ucode minimum), into a staging tile
    that GpSimd converts to bf16. This replaces the 768 tiny
    indirect-DMA calls of the naive version (~1us serialized descriptor-
    generation overhead each) with 48 large ones.
  - scoresT[k,q] = keys @ q.T via PE in bf16, contracting K=64 directly
  - E = exp(scoresT/8) on ScalarE directly PSUM->SBUF (no max-subtraction
    needed: |scores| <= ~14 in fp32)
  - O = E.T-weighted sum of cu, accumulated on PE in PSUM over each chunk of
    8 k-tiles, then folded into an SBUF fp32 accumulator by DVE
  - rowsums r = E.T @ ones accumulated in a persistent PSUM bank
  - out = relu(O @ W) * (1/r), with the 1/r per-partition scale fused into
    the final ReLU PSUM->SBUF copy (valid since r > 0)

Column layout of the gathered cu is [review slots 0-7 | sec slots 0-7]
(instead of the reference's interleaved layout); the host permutes W's rows
to match, so results are identical.
"""

import os
import sys

import ml_dtypes
import numpy as np

for _p in ("/opt/trn_rl_repo", "/root/.axon_site/_ro/trn_rl_repo"):
    if os.path.isdir(_p) and _p not in sys.path:
        sys.path.append(_p)

import concourse.bass as bass  # noqa: E402
import concourse.mybir as mybir  # noqa: E402
import concourse.tile as tile  # noqa: E402
from concourse import bacc  # noqa: E402
from concourse.bass_utils import run_bass_kernel_spmd  # noqa: E402
from concourse.masks import make_identity  # noqa: E402

P = 128
D = 64
NK = 6000          # keys per side
NKP = 6144         # padded to 48 full k-tiles
KT = NKP // P      # 48
NPAIR = KT // 2    # 24 gather pairs
KT_CALC = 47       # k-tiles that carry real keys (kt 47 is all padding)
QOUT = 1500        # query rows per core (6000 / 4 cores per side)
QP = 1536          # padded to 12 full q-subtiles
NQS = QP // P      # 12
HID = 1024
NR = 30000         # review_vecs rows
NS = 6000          # secondary source rows
NIDX = 2048        # indices per dma_gather call (2 k-tiles x 8 slots x 128)
# chunk sizes (even; the gather works in pairs of k-tiles); small first
# chunk so the PE pipeline fills early
CHUNK_SIZES = tuple(int(x) for x in os.environ.get("K_CHUNKS", "4,8,8,8,8,8,4").split(","))
assert sum(CHUNK_SIZES) == 48 and all(c % 2 == 0 for c in CHUNK_SIZES)
CHUNK_STARTS = tuple(int(np.cumsum((0,) + CHUNK_SIZES)[i]) for i in range(len(CHUNK_SIZES)))
NQUEUES = int(os.environ.get("K_QUEUES", "1"))  # SWDGE queues to spread gathers over
F32 = mybir.dt.float32
BF16 = mybir.dt.bfloat16
I32 = mybir.dt.int32

AF = mybir.ActivationFunctionType


def _emit_body(nc, tc, ctx_pools, tensors):
    """Emit one full pass of the kernel body inside an open TileContext."""
    from contextlib import ExitStack

    keysT, qvT, adj, src_r, src_s, w, ebias, out = tensors
    const, psum, psum_r = ctx_pools

    # ---- persistent tiles -------------------------------------------------
    ones = const.tile([P, 1], BF16, tag="ones")
    nc.gpsimd.memset(ones[:], 1.0)

    # only D=64 partitions: the scores matmul contracts over K=64 directly
    # (PE cost depends on moving columns, not K), so no zero-padding needed.
    vecsT = const.tile([D, NKP], BF16, tag="vecsT")
    nc.sync.dma_start(vecsT[:, :], keysT[:, :])

    qvT_sb = const.tile([D, QP], BF16, tag="qvT")
    nc.sync.dma_start(qvT_sb[:, :], qvT[:, :])

    adj_sb = const.tile([P, KT, 16], I32, tag="adj")
    nc.sync.dma_start(adj_sb[:], adj[:, :, :])

    ebias_sb = const.tile([P, 1], F32, tag="ebias")
    nc.sync.dma_start(ebias_sb[:], ebias[:, :])

    # weights for phase B: load early so the DMA overlaps phase A compute
    w_sb = const.tile([P, HID // P, HID], BF16, tag="w")
    nc.sync.dma_start(w_sb[:], w[:, :, :])

    # O accumulated TRANSPOSED: partitions = hid-in (8 chunks of 128), free = q.
    # This feeds phase B's W-matmul lhsT directly - no PE transposes needed.
    o_accT = const.tile([P, HID // P, QP], F32, tag="oaccT")
    obf = const.tile([P, HID // P, QP], BF16, tag="obf")
    r_acc = const.tile([P, NQS], F32, tag="racc")
    rinv = const.tile([P, NQS], F32, tag="rinv")

    chunks = [list(range(st, min(st + cs, KT_CALC)))
              for st, cs in zip(CHUNK_STARTS, CHUNK_SIZES)]

    # ---- phase A: attention numerator + rowsums ---------------------------
    with ExitStack() as ctx:
        e_pool = ctx.enter_context(tc.tile_pool(name="e_pool", bufs=2))
        g_pool = ctx.enter_context(tc.tile_pool(name="g_pool", bufs=2))

        for ci, chunk in enumerate(chunks):
            first_chunk = ci == 0
            st = CHUNK_STARTS[ci]
            n = len(chunk)

            # gathered cu for the chunk: plane 0 = review slots (cu cols
            # 0..511 per k-tile), plane 1 = secondary slots (cols 512..1023);
            # blocks of 64 elems, 8 per k-tile per plane. One indirect DMA
            # per (k-tile, slot): the only index shape walrus lowers
            # correctly is one index per partition.
            g = g_pool.tile([P, 2, n * 8, 64], BF16, tag="g")
            for t, kt in enumerate(chunk):
                for c in range(8):
                    i0 = nc.gpsimd.indirect_dma_start(
                        out=g[:, 0, t * 8 + c, :],
                        out_offset=None,
                        in_=src_r[:],
                        in_offset=bass.IndirectOffsetOnAxis(
                            ap=adj_sb[:, kt, c:c + 1], axis=0),
                    )
                    i1 = nc.gpsimd.indirect_dma_start(
                        out=g[:, 1, t * 8 + c, :],
                        out_offset=None,
                        in_=src_s[:],
                        in_offset=bass.IndirectOffsetOnAxis(
                            ap=adj_sb[:, kt, 8 + c:9 + c], axis=0),
                    )
                    if NQUEUES > 1:
                        q = ((kt * 8 + c) * 2) % NQUEUES
                        i0.queue = f"qPoolDynamic{q or ''}"
                        q = ((kt * 8 + c) * 2 + 1) % NQUEUES
                        i1.queue = f"qPoolDynamic{q or ''}"

            e = e_pool.tile([P, n, QP], BF16, tag="e")
            for t, kt in enumerate(chunk):
                lhsT = vecsT[:, kt * P:(kt + 1) * P]
                for i in range(QP // 512):
                    s_ps = psum.tile([P, 512], F32, tag="ps")
                    nc.tensor.matmul(
                        s_ps[:], lhsT, qvT_sb[:, i * 512:(i + 1) * 512],
                        start=True, stop=True,
                    )
                    # padded key rows (6000..6015) get bias -1e30 so
                    # exp() forces their attention weight to exactly zero
                    bias = ebias_sb[:, 0:1] if kt == KT_CALC - 1 else 0.0
                    nc.scalar.activation(
                        e[:, t, i * 512:(i + 1) * 512], s_ps[:], AF.Exp,
                        bias=bias, scale=0.125,
                    )

            r_ps = psum_r.tile([P, NQS], F32, tag="rps")
            for j in range(NQS):
                for t, kt in enumerate(chunk):
                    nc.tensor.matmul(r_ps[:, j:j + 1], e[:, t, j * P:(j + 1) * P],
                                     ones[:], start=(t == 0), stop=(t == n - 1))
            if first_chunk:
                nc.vector.tensor_copy(r_acc[:], r_ps[:])
            else:
                nc.vector.tensor_add(r_acc[:], r_acc[:], r_ps[:])

            # O.T += g.T @ E per (hid-128-chunk, q-512-block): g is the
            # stationary operand, E the moving one
            for h in range(HID // P):
                plane, hc = divmod(h, 4)
                for qb in range(QP // 512):
                    pt = psum.tile([P, 512], F32, tag="ps")
                    for t, kt in enumerate(chunk):
                        lhsT = g[:, plane, t * 8 + hc * 2:t * 8 + hc * 2 + 2, :]
                        nc.tensor.matmul(pt[:], lhsT,
                                         e[:, t, qb * 512:(qb + 1) * 512],
                                         start=(t == 0), stop=(t == n - 1))
                    dst = o_accT[:, h, qb * 512:(qb + 1) * 512]
                    if first_chunk:
                        nc.vector.tensor_copy(dst, pt[:])
                    else:
                        nc.vector.tensor_add(dst, dst, pt[:])

    # ---- phase B: normalize (folded), project through W, relu, store ------
    nc.vector.reciprocal(rinv[:], r_acc[:])
    for h in range(HID // P):
        nc.vector.tensor_copy(obf[:, h, :], o_accT[:, h, :])

    with ExitStack() as ctx:
        ob_pool = ctx.enter_context(tc.tile_pool(name="ob_pool", bufs=4))

        for j in range(NQS):
            for h in range(HID // 512):
                pf = psum.tile([P, 512], F32, tag="ps")
                for t in range(HID // P):
                    nc.tensor.matmul(
                        pf[:], obf[:, t, j * P:(j + 1) * P],
                        w_sb[:, t, h * 512:(h + 1) * 512],
                        start=(t == 0), stop=(t == HID // P - 1),
                    )
                ob = ob_pool.tile([P, 512], F32, tag="ob")
                nc.scalar.activation(ob[:], pf[:], AF.Relu, scale=rinv[:, j:j + 1])
                rows = min(P, QOUT - j * P)
                if rows > 0:
                    nc.sync.dma_start(
                        out[j * P:j * P + rows, h * 512:(h + 1) * 512], ob[:rows, :],
                    )


def build_program(repeat: int = 0, scratch: int | None = None):
    """Build + compile the SPMD program. repeat>0 wraps the body in a
    device-side For loop (for timing) and is not used for grading."""
    from contextlib import ExitStack

    kw = {} if scratch is None else dict(dynamic_dma_scratch_size=scratch)
    if NQUEUES > 1:
        kw["num_swdge_queues"] = NQUEUES
    nc = bacc.Bacc("TRN2", target_bir_lowering=False, debug=False, num_devices=8, **kw)

    keysT = nc.dram_tensor("keysT", [D, NKP], BF16, kind="ExternalInput")
    qvT = nc.dram_tensor("qvT", [D, QP], BF16, kind="ExternalInput")
    adj = nc.dram_tensor("adj", [P, KT, 16], I32, kind="ExternalInput")
    src_r = nc.dram_tensor("src_r", [NR, D], BF16, kind="ExternalInput")
    src_s = nc.dram_tensor("src_s", [NS, D], BF16, kind="ExternalInput")
    w = nc.dram_tensor("w", [P, HID // P, HID], BF16, kind="ExternalInput")
    ebias = nc.dram_tensor("ebias", [P, 1], F32, kind="ExternalInput")
    out = nc.dram_tensor("out", [QOUT, HID], F32, kind="ExternalOutput")

    tensors = (keysT, qvT, adj, src_r, src_s, w, ebias, out)

    with tile.TileContext(nc) as tc, ExitStack() as ctx:
        const = ctx.enter_context(tc.tile_pool(name="const", bufs=1))
        psum = ctx.enter_context(tc.tile_pool(name="psum", bufs=6, space="PSUM"))
        psum_r = ctx.enter_context(tc.tile_pool(name="psum_r", bufs=2, space="PSUM"))
        pools = (const, psum, psum_r)
        for _ in range(max(repeat, 1)):
            _emit_body(nc, tc, pools, tensors)

    nc.compile()
    return nc


def _permute_w(w_full: np.ndarray) -> np.ndarray:
    """Reference cu columns are slot-interleaved [r0 i0 r1 i1 ...]; the kernel
    gathers [r0..r7 | i0..i7]. Permute W rows to match, then pre-tile to
    [128, 8, 1024] for the on-device layout."""
    wr = w_full.reshape(8, 2, D, HID)
    w_perm = np.concatenate(
        [wr[:, 0].reshape(8 * D, HID), wr[:, 1].reshape(8 * D, HID)], axis=0,
    )
    return np.ascontiguousarray(
        w_perm.reshape(HID // P, P, HID).transpose(1, 0, 2),
    )


def _merge_adj(adj_r: np.ndarray, adj_s: np.ndarray) -> np.ndarray:
    """[6000, 8] x2 -> [128, KT, 16] int32: per (partition, k-tile) the 16
    indices are [r0..r7, s0..s7]. Padded rows index row 0 (their attention
    weight is forced to zero)."""
    a = np.zeros((NKP, 16), dtype=np.int32)
    a[:NK, 0:8] = adj_r
    a[:NK, 8:16] = adj_s
    return np.ascontiguousarray(a.reshape(KT, P, 16).transpose(1, 0, 2))


def _host_inputs(review_vecs, user_vecs, item_vecs, user_weights, item_weights,
                 user_review_adj, user_item_adj, item_review_adj, item_user_adj):
    review_vecs = np.asarray(review_vecs, dtype=np.float32)
    user_vecs = np.asarray(user_vecs, dtype=np.float32)
    item_vecs = np.asarray(item_vecs, dtype=np.float32)
    review_b = review_vecs.astype(ml_dtypes.bfloat16)

    sides = {}
    for side, keys, adj_r, adj_s, src_s, w_full in (
        ("user", user_vecs, user_review_adj, user_item_adj, item_vecs, user_weights),
        ("item", item_vecs, item_review_adj, item_user_adj, user_vecs, item_weights),
    ):
        keysT = np.zeros((D, NKP), dtype=ml_dtypes.bfloat16)
        keysT[:, :NK] = keys.T.astype(ml_dtypes.bfloat16)
        sides[side] = dict(
            keysT=keysT,
            adj=_merge_adj(np.asarray(adj_r, dtype=np.int32),
                           np.asarray(adj_s, dtype=np.int32)),
            src_s=np.ascontiguousarray(np.asarray(src_s).astype(ml_dtypes.bfloat16)),
            w=_permute_w(np.asarray(w_full, dtype=np.float32)).astype(ml_dtypes.bfloat16),
            keys=keys,
        )

    ebias = np.zeros((P, 1), dtype=np.float32)
    ebias[NK - (KT_CALC - 1) * P:] = -1e30

    in_maps = []
    for c in range(8):
        s = sides["user" if c < 4 else "item"]
        b = c % 4
        qv = s["keys"][b * QOUT:(b + 1) * QOUT].astype(ml_dtypes.bfloat16)  # [1500, 64]
        qvT = np.empty((D, QP), dtype=ml_dtypes.bfloat16)
        qvT[:, :QOUT] = qv.T
        qvT[:, QOUT:] = qv.T[:, :QP - QOUT]  # pad with real vectors (finite rowsums)
        in_maps.append(dict(
            keysT=s["keysT"], qvT=np.ascontiguousarray(qvT),
            adj=s["adj"], src_r=review_b, src_s=s["src_s"],
            w=s["w"], ebias=ebias,
        ))
    return in_maps


_NC_CACHE = None


def kernel(**inputs):
    global _NC_CACHE
    if _NC_CACHE is None:
        _NC_CACHE = build_program()
    nc = _NC_CACHE
    in_maps = _host_inputs(**inputs)
    res = run_bass_kernel_spmd(nc, in_maps, core_ids=list(range(8)))
    outs = [res.results[c]["out"] for c in range(8)]
    user_output = np.concatenate(outs[0:4], axis=0)
    item_output = np.concatenate(outs[4:8], axis=0)
    return user_output, item_output


# revision 32
# speedup vs baseline: 1.4094x; 1.2374x over previous
"""Trainium2 Bass kernel for nn_AttentionAggregator.

Computation (per side, users/items symmetric):
    cu  = concat(gather(review_vecs, adj_r), gather(sec_vecs, adj_s))   # [6000, 1024]
    att = softmax(keys @ keys.T / 8) @ cu                               # [6000, 1024]
    out = relu(att @ W)                                                 # [6000, 1024]

Sharding: 8 cores run the same program (SPMD). Cores 0-3 take the user side
(1500 query rows each), cores 4-7 the item side. Keys, gather sources,
adjacency and weights are replicated; only the query slice differs.

On-device per core:
  - neighbor features are gathered from DRAM by indirect DMA, one call per
    (k-tile, slot, source) with a [128, 1] index column (one row per
    partition) - the only index shape the walrus indirect-DMA lowering
    handles correctly on hardware (multi-column index APs scramble the
    routing, and the InstDMAGatherAnt extended-ucode library is not loadable
    on this terminal). The 768 calls pipeline behind PE compute.
  - scoresT[k,q] = keys @ q.T via PE in bf16, contracting K=64 directly
  - E = exp(scoresT/8) on ScalarE directly PSUM->SBUF (no max-subtraction
    needed: |scores| <= ~14 in fp32)
  - O.T = cu.T-weighted sum of E, accumulated on PE in PSUM over each chunk
    of 8 k-tiles with the gathered cu as the stationary operand, then folded
    into an SBUF fp32 accumulator by DVE. Accumulating O TRANSPOSED feeds
    phase B's W-matmul directly (no PE transposes needed).
  - rowsums r = E.T @ ones accumulated in a persistent PSUM bank
  - out = relu(O @ W) * (1/r), with the 1/r per-partition scale fused into
    the final ReLU PSUM->SBUF copy (valid since r > 0)

Column layout of the gathered cu is [review slots 0-7 | sec slots 0-7]
(instead of the reference's interleaved layout); the host permutes W's rows
to match, so results are identical.
"""

import os
import sys

import ml_dtypes
import numpy as np

for _p in ("/opt/trn_rl_repo", "/root/.axon_site/_ro/trn_rl_repo"):
    if os.path.isdir(_p) and _p not in sys.path:
        sys.path.append(_p)

import concourse.bass as bass  # noqa: E402
import concourse.mybir as mybir  # noqa: E402
import concourse.tile as tile  # noqa: E402
from concourse import bacc  # noqa: E402
from concourse.bass_utils import run_bass_kernel_spmd  # noqa: E402

P = 128
D = 64
NK = 6000          # keys per side
NKP = 6144         # padded to 48 full k-tiles
KT = NKP // P      # 48
KT_CALC = 47       # k-tiles that carry real keys (kt 47 is all padding)
QOUT = 1500        # query rows per core (6000 / 4 cores per side)
QP = 1536          # padded to 12 full q-subtiles
NQS = QP // P      # 12
HID = 1024
NR = 30000         # review_vecs rows
NS = 6000          # secondary source rows
# chunk sizes (even; the gather works in pairs of k-tiles); small first
# chunk so the PE pipeline fills early
CHUNK_SIZES = tuple(int(x) for x in os.environ.get("K_CHUNKS", "4,8,8,8,8,8,4").split(","))
assert sum(CHUNK_SIZES) == 48 and all(c % 2 == 0 for c in CHUNK_SIZES)
CHUNK_STARTS = tuple(int(np.cumsum((0,) + CHUNK_SIZES)[i]) for i in range(len(CHUNK_SIZES)))
NQUEUES = int(os.environ.get("K_QUEUES", "1"))  # SWDGE queues to spread gathers over
F32 = mybir.dt.float32
BF16 = mybir.dt.bfloat16
I32 = mybir.dt.int32

AF = mybir.ActivationFunctionType


def _emit_body(nc, tc, ctx_pools, tensors):
    """Emit one full pass of the kernel body inside an open TileContext."""
    from contextlib import ExitStack

    keysT, qvT, adj, src_r, src_s, w, ebias, out = tensors
    const, psum, psum_r = ctx_pools

    # ---- persistent tiles -------------------------------------------------
    ones = const.tile([P, 1], BF16, tag="ones")
    nc.gpsimd.memset(ones[:], 1.0)

    # only D=64 partitions: the scores matmul contracts over K=64 directly
    # (PE cost depends on moving columns, not K), so no zero-padding needed.
    vecsT = const.tile([D, NKP], BF16, tag="vecsT")
    nc.sync.dma_start(vecsT[:, :], keysT[:, :])

    qvT_sb = const.tile([D, QP], BF16, tag="qvT")
    nc.sync.dma_start(qvT_sb[:, :], qvT[:, :])

    adj_sb = const.tile([P, KT, 16], I32, tag="adj")
    nc.sync.dma_start(adj_sb[:], adj[:, :, :])

    ebias_sb = const.tile([P, 1], F32, tag="ebias")
    nc.sync.dma_start(ebias_sb[:], ebias[:, :])

    # weights for phase B: load early so the DMA overlaps phase A compute
    w_sb = const.tile([P, HID // P, HID], BF16, tag="w")
    nc.sync.dma_start(w_sb[:], w[:, :, :])

    # O accumulated TRANSPOSED: partitions = hid-in (8 chunks of 128), free = q.
    # This feeds phase B's W-matmul lhsT directly - no PE transposes needed.
    o_accT = const.tile([P, HID // P, QP], F32, tag="oaccT")
    obf = const.tile([P, HID // P, QP], BF16, tag="obf")
    r_acc = const.tile([P, NQS], F32, tag="racc")
    rinv = const.tile([P, NQS], F32, tag="rinv")

    chunks = [list(range(st, min(st + cs, KT_CALC)))
              for st, cs in zip(CHUNK_STARTS, CHUNK_SIZES)]

    # ---- phase A: attention numerator + rowsums ---------------------------
    with ExitStack() as ctx:
        e_pool = ctx.enter_context(tc.tile_pool(name="e_pool", bufs=2))
        g_pool = ctx.enter_context(tc.tile_pool(name="g_pool", bufs=2))

        for ci, chunk in enumerate(chunks):
            first_chunk = ci == 0
            st = CHUNK_STARTS[ci]
            n = len(chunk)

            # gathered cu for the chunk: plane 0 = review slots (cu cols
            # 0..511 per k-tile), plane 1 = secondary slots (cols 512..1023);
            # blocks of 64 elems, 8 per k-tile per plane. One indirect DMA
            # per (k-tile, slot): the only index shape walrus lowers
            # correctly is one index per partition.
            g = g_pool.tile([P, 2, n * 8, 64], BF16, tag="g")
            for t, kt in enumerate(chunk):
                for c in range(8):
                    i0 = nc.gpsimd.indirect_dma_start(
                        out=g[:, 0, t * 8 + c, :],
                        out_offset=None,
                        in_=src_r[:],
                        in_offset=bass.IndirectOffsetOnAxis(
                            ap=adj_sb[:, kt, c:c + 1], axis=0),
                    )
                    i1 = nc.gpsimd.indirect_dma_start(
                        out=g[:, 1, t * 8 + c, :],
                        out_offset=None,
                        in_=src_s[:],
                        in_offset=bass.IndirectOffsetOnAxis(
                            ap=adj_sb[:, kt, 8 + c:9 + c], axis=0),
                    )
                    if NQUEUES > 1:
                        q = ((kt * 8 + c) * 2) % NQUEUES
                        i0.queue = f"qPoolDynamic{q or ''}"
                        q = ((kt * 8 + c) * 2 + 1) % NQUEUES
                        i1.queue = f"qPoolDynamic{q or ''}"

            e = e_pool.tile([P, n, QP], BF16, tag="e")
            for t, kt in enumerate(chunk):
                lhsT = vecsT[:, kt * P:(kt + 1) * P]
                for i in range(QP // 512):
                    s_ps = psum.tile([P, 512], F32, tag="ps")
                    nc.tensor.matmul(
                        s_ps[:], lhsT, qvT_sb[:, i * 512:(i + 1) * 512],
                        start=True, stop=True,
                    )
                    # padded key rows (6000..6015) get bias -1e30 so
                    # exp() forces their attention weight to exactly zero
                    bias = ebias_sb[:, 0:1] if kt == KT_CALC - 1 else 0.0
                    nc.scalar.activation(
                        e[:, t, i * 512:(i + 1) * 512], s_ps[:], AF.Exp,
                        bias=bias, scale=0.125,
                    )

            r_ps = psum_r.tile([P, NQS], F32, tag="rps")
            for j in range(NQS):
                for t, kt in enumerate(chunk):
                    nc.tensor.matmul(r_ps[:, j:j + 1], e[:, t, j * P:(j + 1) * P],
                                     ones[:], start=(t == 0), stop=(t == n - 1))
            if first_chunk:
                nc.vector.tensor_copy(r_acc[:], r_ps[:])
            else:
                nc.vector.tensor_add(r_acc[:], r_acc[:], r_ps[:])

            # O.T += g.T @ E per (hid-128-chunk, q-512-block): g is the
            # stationary operand, E the moving one
            for h in range(HID // P):
                plane, hc = divmod(h, 4)
                for qb in range(QP // 512):
                    pt = psum.tile([P, 512], F32, tag="ps")
                    for t, kt in enumerate(chunk):
                        lhsT = g[:, plane, t * 8 + hc * 2:t * 8 + hc * 2 + 2, :]
                        nc.tensor.matmul(pt[:], lhsT,
                                         e[:, t, qb * 512:(qb + 1) * 512],
                                         start=(t == 0), stop=(t == n - 1))
                    dst = o_accT[:, h, qb * 512:(qb + 1) * 512]
                    if first_chunk:
                        nc.vector.tensor_copy(dst, pt[:])
                    else:
                        nc.vector.tensor_add(dst, dst, pt[:])

    # ---- phase B: normalize (folded), project through W, relu, store ------
    nc.vector.reciprocal(rinv[:], r_acc[:])
    for h in range(HID // P):
        nc.vector.tensor_copy(obf[:, h, :], o_accT[:, h, :])

    with ExitStack() as ctx:
        ob_pool = ctx.enter_context(tc.tile_pool(name="ob_pool", bufs=4))

        for j in range(NQS):
            for h in range(HID // 512):
                pf = psum.tile([P, 512], F32, tag="ps")
                for t in range(HID // P):
                    nc.tensor.matmul(
                        pf[:], obf[:, t, j * P:(j + 1) * P],
                        w_sb[:, t, h * 512:(h + 1) * 512],
                        start=(t == 0), stop=(t == HID // P - 1),
                    )
                ob = ob_pool.tile([P, 512], F32, tag="ob")
                nc.scalar.activation(ob[:], pf[:], AF.Relu, scale=rinv[:, j:j + 1])
                rows = min(P, QOUT - j * P)
                if rows > 0:
                    nc.sync.dma_start(
                        out[j * P:j * P + rows, h * 512:(h + 1) * 512], ob[:rows, :],
                    )


def build_program(repeat: int = 0, scratch: int | None = None):
    """Build + compile the SPMD program. repeat>0 wraps the body in a
    device-side For loop (for timing) and is not used for grading."""
    from contextlib import ExitStack

    kw = {} if scratch is None else dict(dynamic_dma_scratch_size=scratch)
    if NQUEUES > 1:
        kw["num_swdge_queues"] = NQUEUES
    nc = bacc.Bacc("TRN2", target_bir_lowering=False, debug=False, num_devices=8, **kw)

    keysT = nc.dram_tensor("keysT", [D, NKP], BF16, kind="ExternalInput")
    qvT = nc.dram_tensor("qvT", [D, QP], BF16, kind="ExternalInput")
    adj = nc.dram_tensor("adj", [P, KT, 16], I32, kind="ExternalInput")
    src_r = nc.dram_tensor("src_r", [NR, D], BF16, kind="ExternalInput")
    src_s = nc.dram_tensor("src_s", [NS, D], BF16, kind="ExternalInput")
    w = nc.dram_tensor("w", [P, HID // P, HID], BF16, kind="ExternalInput")
    ebias = nc.dram_tensor("ebias", [P, 1], F32, kind="ExternalInput")
    out = nc.dram_tensor("out", [QOUT, HID], F32, kind="ExternalOutput")

    tensors = (keysT, qvT, adj, src_r, src_s, w, ebias, out)

    with tile.TileContext(nc) as tc, ExitStack() as ctx:
        const = ctx.enter_context(tc.tile_pool(name="const", bufs=1))
        psum = ctx.enter_context(tc.tile_pool(name="psum", bufs=6, space="PSUM"))
        psum_r = ctx.enter_context(tc.tile_pool(name="psum_r", bufs=2, space="PSUM"))
        pools = (const, psum, psum_r)
        for _ in range(max(repeat, 1)):
            _emit_body(nc, tc, pools, tensors)

    nc.compile()
    return nc


def _permute_w(w_full: np.ndarray) -> np.ndarray:
    """Reference cu columns are slot-interleaved [r0 i0 r1 i1 ...]; the kernel
    gathers [r0..r7 | i0..i7]. Permute W rows to match, then pre-tile to
    [128, 8, 1024] for the on-device layout."""
    wr = w_full.reshape(8, 2, D, HID)
    w_perm = np.concatenate(
        [wr[:, 0].reshape(8 * D, HID), wr[:, 1].reshape(8 * D, HID)], axis=0,
    )
    return np.ascontiguousarray(
        w_perm.reshape(HID // P, P, HID).transpose(1, 0, 2),
    )


def _merge_adj(adj_r: np.ndarray, adj_s: np.ndarray) -> np.ndarray:
    """[6000, 8] x2 -> [128, KT, 16] int32: per (partition, k-tile) the 16
    indices are [r0..r7, s0..s7]. Padded rows index row 0 (their attention
    weight is forced to zero)."""
    a = np.zeros((NKP, 16), dtype=np.int32)
    a[:NK, 0:8] = adj_r
    a[:NK, 8:16] = adj_s
    return np.ascontiguousarray(a.reshape(KT, P, 16).transpose(1, 0, 2))


def _host_inputs(review_vecs, user_vecs, item_vecs, user_weights, item_weights,
                 user_review_adj, user_item_adj, item_review_adj, item_user_adj):
    review_vecs = np.asarray(review_vecs, dtype=np.float32)
    user_vecs = np.asarray(user_vecs, dtype=np.float32)
    item_vecs = np.asarray(item_vecs, dtype=np.float32)
    review_b = review_vecs.astype(ml_dtypes.bfloat16)

    sides = {}
    for side, keys, adj_r, adj_s, src_s, w_full in (
        ("user", user_vecs, user_review_adj, user_item_adj, item_vecs, user_weights),
        ("item", item_vecs, item_review_adj, item_user_adj, user_vecs, item_weights),
    ):
        keysT = np.zeros((D, NKP), dtype=ml_dtypes.bfloat16)
        keysT[:, :NK] = keys.T.astype(ml_dtypes.bfloat16)
        sides[side] = dict(
            keysT=keysT,
            adj=_merge_adj(np.asarray(adj_r, dtype=np.int32),
                           np.asarray(adj_s, dtype=np.int32)),
            src_s=np.ascontiguousarray(np.asarray(src_s).astype(ml_dtypes.bfloat16)),
            w=_permute_w(np.asarray(w_full, dtype=np.float32)).astype(ml_dtypes.bfloat16),
            keys=keys,
        )

    ebias = np.zeros((P, 1), dtype=np.float32)
    ebias[NK - (KT_CALC - 1) * P:] = -1e30

    in_maps = []
    for c in range(8):
        s = sides["user" if c < 4 else "item"]
        b = c % 4
        qv = s["keys"][b * QOUT:(b + 1) * QOUT].astype(ml_dtypes.bfloat16)  # [1500, 64]
        qvT = np.empty((D, QP), dtype=ml_dtypes.bfloat16)
        qvT[:, :QOUT] = qv.T
        qvT[:, QOUT:] = qv.T[:, :QP - QOUT]  # pad with real vectors (finite rowsums)
        in_maps.append(dict(
            keysT=s["keysT"], qvT=np.ascontiguousarray(qvT),
            adj=s["adj"], src_r=review_b, src_s=s["src_s"],
            w=s["w"], ebias=ebias,
        ))
    return in_maps


_NC_CACHE = None


def kernel(**inputs):
    global _NC_CACHE
    if _NC_CACHE is None:
        _NC_CACHE = build_program()
    nc = _NC_CACHE
    in_maps = _host_inputs(**inputs)
    res = run_bass_kernel_spmd(nc, in_maps, core_ids=list(range(8)))
    outs = [res.results[c]["out"] for c in range(8)]
    user_output = np.concatenate(outs[0:4], axis=0)
    item_output = np.concatenate(outs[4:8], axis=0)
    return user_output, item_output


# revision 47
# speedup vs baseline: 1.5816x; 1.1222x over previous
"""Trainium2 Bass kernel for nn_AttentionAggregator.

Computation (per side, users/items symmetric):
    cu  = concat(gather(review_vecs, adj_r), gather(sec_vecs, adj_s))   # [6000, 1024]
    att = softmax(keys @ keys.T / 8) @ cu                               # [6000, 1024]
    out = relu(att @ W)                                                 # [6000, 1024]

Sharding: 8 cores run the same program (SPMD). Cores 0-3 take the user side
(1500 query rows each), cores 4-7 the item side. Keys, gather sources,
adjacency and weights are replicated; only the query slice differs.

On-device per core:
  - neighbor features are gathered from DRAM by indirect DMA, one call per
    (k-tile, slot, source) with a [128, 1] index column (one row per
    partition) - the only index shape the walrus indirect-DMA lowering
    handles correctly on hardware (multi-column index APs scramble the
    routing, and the InstDMAGatherAnt extended-ucode library is not loadable
    on this terminal). The 768 calls pipeline behind PE compute.
  - scoresT[k,q] = keys @ q.T via PE in bf16, contracting K=64 directly
  - E = exp(scoresT/8) on ScalarE directly PSUM->SBUF (no max-subtraction
    needed: |scores| <= ~14 in fp32)
  - O.T = cu.T-weighted sum of E, accumulated on PE in PSUM over each chunk
    of 8 k-tiles with the gathered cu as the stationary operand, then folded
    into an SBUF fp32 accumulator by DVE. Accumulating O TRANSPOSED feeds
    phase B's W-matmul directly (no PE transposes needed).
  - rowsums r = E.T @ ones accumulated in a persistent PSUM bank
  - out = relu(O @ W) * (1/r), with the 1/r per-partition scale fused into
    the final ReLU PSUM->SBUF copy (valid since r > 0)

Column layout of the gathered cu is [review slots 0-7 | sec slots 0-7]
(instead of the reference's interleaved layout); the host permutes W's rows
to match, so results are identical.
"""

import os
import sys

import ml_dtypes
import numpy as np

for _p in ("/opt/trn_rl_repo", "/root/.axon_site/_ro/trn_rl_repo"):
    if os.path.isdir(_p) and _p not in sys.path:
        sys.path.append(_p)

import concourse.bass as bass  # noqa: E402
import concourse.mybir as mybir  # noqa: E402
import concourse.tile as tile  # noqa: E402
from concourse import bacc  # noqa: E402
from concourse.bass_utils import run_bass_kernel_spmd  # noqa: E402

P = 128
D = 64
NK = 6000          # keys per side
NKP = 6144         # padded to 48 full k-tiles
KT = NKP // P      # 48
KT_CALC = 47       # k-tiles that carry real keys (kt 47 is all padding)
QOUT = 1500        # query rows per core (6000 / 4 cores per side); NOT padded
NQS = 12           # q-subtiles of 128 (last one holds only 92 real queries)
QB = ((0, 512), (512, 1024), (1024, 1500))   # moving-operand q blocks
HID = 1024
NR = 30000         # review_vecs rows
NS = 6000          # secondary source rows
# chunk sizes (even; the gather works in pairs of k-tiles); small first
# chunk so the PE pipeline fills early
CHUNK_SIZES = tuple(int(x) for x in os.environ.get("K_CHUNKS", "4,8,8,8,8,8,4").split(","))
assert sum(CHUNK_SIZES) == 48 and all(c % 2 == 0 for c in CHUNK_SIZES)
CHUNK_STARTS = tuple(int(np.cumsum((0,) + CHUNK_SIZES)[i]) for i in range(len(CHUNK_SIZES)))
NQUEUES = int(os.environ.get("K_QUEUES", "1"))  # SWDGE queues to spread gathers over
F32 = mybir.dt.float32
BF16 = mybir.dt.bfloat16
I32 = mybir.dt.int32

AF = mybir.ActivationFunctionType


def _emit_body(nc, tc, ctx_pools, tensors):
    """Emit one full pass of the kernel body inside an open TileContext."""
    from contextlib import ExitStack

    keysT, qvT, adj, src_r, src_s, w, ebias, out = tensors
    const, psum, psum_r = ctx_pools

    # ---- persistent tiles -------------------------------------------------
    ones = const.tile([P, 1], BF16, tag="ones")
    nc.gpsimd.memset(ones[:], 1.0)

    # only D=64 partitions: the scores matmul contracts over K=64 directly
    # (PE cost depends on moving columns, not K), so no zero-padding needed.
    vecsT = const.tile([D, NKP], BF16, tag="vecsT")
    nc.sync.dma_start(vecsT[:, :], keysT[:, :])

    qvT_sb = const.tile([D, QOUT], BF16, tag="qvT")
    nc.sync.dma_start(qvT_sb[:, :], qvT[:, :])

    adj_sb = const.tile([P, KT, 16], I32, tag="adj")
    nc.sync.dma_start(adj_sb[:], adj[:, :, :])

    ebias_sb = const.tile([P, 1], F32, tag="ebias")
    nc.sync.dma_start(ebias_sb[:], ebias[:, :])

    # weights for phase B: load early so the DMA overlaps phase A compute
    w_sb = const.tile([P, HID // P, HID], BF16, tag="w")
    nc.sync.dma_start(w_sb[:], w[:, :, :])

    # O accumulated TRANSPOSED: partitions = hid-in (8 chunks of 128), free = q.
    # This feeds phase B's W-matmul lhsT directly - no PE transposes needed.
    o_accT = const.tile([P, HID // P, QOUT], F32, tag="oaccT")
    obf = const.tile([P, HID // P, QOUT], BF16, tag="obf")
    r_acc = const.tile([P, NQS], F32, tag="racc")
    rinv = const.tile([P, NQS], F32, tag="rinv")

    chunks = [list(range(st, min(st + cs, KT_CALC)))
              for st, cs in zip(CHUNK_STARTS, CHUNK_SIZES)]

    # ---- phase A: attention numerator + rowsums ---------------------------
    with ExitStack() as ctx:
        e_pool = ctx.enter_context(tc.tile_pool(name="e_pool", bufs=2))
        g_pool = ctx.enter_context(tc.tile_pool(name="g_pool", bufs=2))

        for ci, chunk in enumerate(chunks):
            first_chunk = ci == 0
            st = CHUNK_STARTS[ci]
            n = len(chunk)

            # gathered cu for the chunk: plane 0 = review slots (cu cols
            # 0..511 per k-tile), plane 1 = secondary slots (cols 512..1023);
            # blocks of 64 elems, 8 per k-tile per plane. One indirect DMA
            # per (k-tile, slot): the only index shape walrus lowers
            # correctly is one index per partition.
            g = g_pool.tile([P, 2, n * 8, 64], BF16, tag="g")
            for t, kt in enumerate(chunk):
                for c in range(8):
                    i0 = nc.gpsimd.indirect_dma_start(
                        out=g[:, 0, t * 8 + c, :],
                        out_offset=None,
                        in_=src_r[:],
                        in_offset=bass.IndirectOffsetOnAxis(
                            ap=adj_sb[:, kt, c:c + 1], axis=0),
                    )
                    i1 = nc.gpsimd.indirect_dma_start(
                        out=g[:, 1, t * 8 + c, :],
                        out_offset=None,
                        in_=src_s[:],
                        in_offset=bass.IndirectOffsetOnAxis(
                            ap=adj_sb[:, kt, 8 + c:9 + c], axis=0),
                    )
                    if NQUEUES > 1:
                        q = ((kt * 8 + c) * 2) % NQUEUES
                        i0.queue = f"qPoolDynamic{q or ''}"
                        q = ((kt * 8 + c) * 2 + 1) % NQUEUES
                        i1.queue = f"qPoolDynamic{q or ''}"

            e = e_pool.tile([P, n, QOUT], BF16, tag="e")
            for t, kt in enumerate(chunk):
                lhsT = vecsT[:, kt * P:(kt + 1) * P]
                for lo, hi in QB:
                    s_ps = psum.tile([P, 512], F32, tag="ps")
                    nc.tensor.matmul(
                        s_ps[:, 0:hi - lo], lhsT, qvT_sb[:, lo:hi],
                        start=True, stop=True,
                    )
                    # padded key rows (6000..6015) get bias -1e30 so
                    # exp() forces their attention weight to exactly zero
                    bias = ebias_sb[:, 0:1] if kt == KT_CALC - 1 else 0.0
                    nc.scalar.activation(
                        e[:, t, lo:hi], s_ps[:, 0:hi - lo], AF.Exp,
                        bias=bias, scale=0.125,
                    )

            r_ps = psum_r.tile([P, NQS], F32, tag="rps")
            for j in range(NQS):
                wj = min(P, QOUT - j * P)
                for t, kt in enumerate(chunk):
                    nc.tensor.matmul(r_ps[0:wj, j:j + 1],
                                     e[:, t, j * P:j * P + wj],
                                     ones[:], start=(t == 0), stop=(t == n - 1))
            # last r_ps column only has QOUT-11*128=92 valid partitions
            for sl in (np.s_[:, 0:NQS - 1], np.s_[0:QOUT - (NQS - 1) * P, NQS - 1:NQS]):
                if first_chunk:
                    nc.vector.tensor_copy(r_acc[sl], r_ps[sl])
                else:
                    nc.vector.tensor_add(r_acc[sl], r_acc[sl], r_ps[sl])

            # O.T += g.T @ E per (hid-128-chunk, q-512-block): g is the
            # stationary operand, E the moving one
            for h in range(HID // P):
                plane, hc = divmod(h, 4)
                for lo, hi in QB:
                    pt = psum.tile([P, 512], F32, tag="ps")
                    for t, kt in enumerate(chunk):
                        lhsT = g[:, plane, t * 8 + hc * 2:t * 8 + hc * 2 + 2, :]
                        nc.tensor.matmul(pt[:, 0:hi - lo], lhsT,
                                         e[:, t, lo:hi],
                                         start=(t == 0), stop=(t == n - 1))
                    dst = o_accT[:, h, lo:hi]
                    if first_chunk:
                        nc.vector.tensor_copy(dst, pt[:, 0:hi - lo])
                    else:
                        nc.vector.tensor_add(dst, dst, pt[:, 0:hi - lo])

    # ---- phase B: normalize (folded), project through W, relu, store ------
    nc.vector.reciprocal(rinv[:, 0:NQS - 1], r_acc[:, 0:NQS - 1])
    nc.vector.reciprocal(rinv[0:QOUT - (NQS - 1) * P, NQS - 1:NQS],
                         r_acc[0:QOUT - (NQS - 1) * P, NQS - 1:NQS])
    for h in range(HID // P):
        nc.vector.tensor_copy(obf[:, h, :], o_accT[:, h, :])

    with ExitStack() as ctx:
        ob_pool = ctx.enter_context(tc.tile_pool(name="ob_pool", bufs=4))

        for j in range(NQS):
            wj = min(P, QOUT - j * P)
            for h in range(HID // 512):
                pf = psum.tile([P, 512], F32, tag="ps")
                for t in range(HID // P):
                    nc.tensor.matmul(
                        pf[0:wj, :], obf[:, t, j * P:j * P + wj],
                        w_sb[:, t, h * 512:(h + 1) * 512],
                        start=(t == 0), stop=(t == HID // P - 1),
                    )
                ob = ob_pool.tile([P, 512], F32, tag="ob")
                nc.scalar.activation(ob[0:wj, :], pf[0:wj, :], AF.Relu,
                                     scale=rinv[0:wj, j:j + 1])
                nc.sync.dma_start(
                    out[j * P:j * P + wj, h * 512:(h + 1) * 512], ob[:wj, :],
                )


def build_program(repeat: int = 0, scratch: int | None = None):
    """Build + compile the SPMD program. repeat>0 wraps the body in a
    device-side For loop (for timing) and is not used for grading."""
    from contextlib import ExitStack

    kw = {} if scratch is None else dict(dynamic_dma_scratch_size=scratch)
    if NQUEUES > 1:
        kw["num_swdge_queues"] = NQUEUES
    nc = bacc.Bacc("TRN2", target_bir_lowering=False, debug=False, num_devices=8, **kw)

    keysT = nc.dram_tensor("keysT", [D, NKP], BF16, kind="ExternalInput")
    qvT = nc.dram_tensor("qvT", [D, QOUT], BF16, kind="ExternalInput")
    adj = nc.dram_tensor("adj", [P, KT, 16], I32, kind="ExternalInput")
    src_r = nc.dram_tensor("src_r", [NR, D], BF16, kind="ExternalInput")
    src_s = nc.dram_tensor("src_s", [NS, D], BF16, kind="ExternalInput")
    w = nc.dram_tensor("w", [P, HID // P, HID], BF16, kind="ExternalInput")
    ebias = nc.dram_tensor("ebias", [P, 1], F32, kind="ExternalInput")
    out = nc.dram_tensor("out", [QOUT, HID], F32, kind="ExternalOutput")

    tensors = (keysT, qvT, adj, src_r, src_s, w, ebias, out)

    with tile.TileContext(nc) as tc, ExitStack() as ctx:
        const = ctx.enter_context(tc.tile_pool(name="const", bufs=1))
        psum = ctx.enter_context(tc.tile_pool(name="psum", bufs=6, space="PSUM"))
        psum_r = ctx.enter_context(tc.tile_pool(name="psum_r", bufs=2, space="PSUM"))
        pools = (const, psum, psum_r)
        for _ in range(max(repeat, 1)):
            _emit_body(nc, tc, pools, tensors)

    nc.compile()
    return nc


def _permute_w(w_full: np.ndarray) -> np.ndarray:
    """Reference cu columns are slot-interleaved [r0 i0 r1 i1 ...]; the kernel
    gathers [r0..r7 | i0..i7]. Permute W rows to match, then pre-tile to
    [128, 8, 1024] for the on-device layout."""
    wr = w_full.reshape(8, 2, D, HID)
    w_perm = np.concatenate(
        [wr[:, 0].reshape(8 * D, HID), wr[:, 1].reshape(8 * D, HID)], axis=0,
    )
    return np.ascontiguousarray(
        w_perm.reshape(HID // P, P, HID).transpose(1, 0, 2),
    )


def _merge_adj(adj_r: np.ndarray, adj_s: np.ndarray) -> np.ndarray:
    """[6000, 8] x2 -> [128, KT, 16] int32: per (partition, k-tile) the 16
    indices are [r0..r7, s0..s7]. Padded rows index row 0 (their attention
    weight is forced to zero)."""
    a = np.zeros((NKP, 16), dtype=np.int32)
    a[:NK, 0:8] = adj_r
    a[:NK, 8:16] = adj_s
    return np.ascontiguousarray(a.reshape(KT, P, 16).transpose(1, 0, 2))


def _host_inputs(review_vecs, user_vecs, item_vecs, user_weights, item_weights,
                 user_review_adj, user_item_adj, item_review_adj, item_user_adj):
    review_vecs = np.asarray(review_vecs, dtype=np.float32)
    user_vecs = np.asarray(user_vecs, dtype=np.float32)
    item_vecs = np.asarray(item_vecs, dtype=np.float32)
    review_b = review_vecs.astype(ml_dtypes.bfloat16)

    sides = {}
    for side, keys, adj_r, adj_s, src_s, w_full in (
        ("user", user_vecs, user_review_adj, user_item_adj, item_vecs, user_weights),
        ("item", item_vecs, item_review_adj, item_user_adj, user_vecs, item_weights),
    ):
        keysT = np.zeros((D, NKP), dtype=ml_dtypes.bfloat16)
        keysT[:, :NK] = keys.T.astype(ml_dtypes.bfloat16)
        sides[side] = dict(
            keysT=keysT,
            adj=_merge_adj(np.asarray(adj_r, dtype=np.int32),
                           np.asarray(adj_s, dtype=np.int32)),
            src_s=np.ascontiguousarray(np.asarray(src_s).astype(ml_dtypes.bfloat16)),
            w=_permute_w(np.asarray(w_full, dtype=np.float32)).astype(ml_dtypes.bfloat16),
            keys=keys,
        )

    ebias = np.zeros((P, 1), dtype=np.float32)
    ebias[NK - (KT_CALC - 1) * P:] = -1e30

    in_maps = []
    for c in range(8):
        s = sides["user" if c < 4 else "item"]
        b = c % 4
        qv = s["keys"][b * QOUT:(b + 1) * QOUT].astype(ml_dtypes.bfloat16)  # [1500, 64]
        qvT = qv.T  # [64, 1500], no padding
        in_maps.append(dict(
            keysT=s["keysT"], qvT=np.ascontiguousarray(qvT),
            adj=s["adj"], src_r=review_b, src_s=s["src_s"],
            w=s["w"], ebias=ebias,
        ))
    return in_maps


_NC_CACHE = None


def kernel(**inputs):
    global _NC_CACHE
    if _NC_CACHE is None:
        _NC_CACHE = build_program()
    nc = _NC_CACHE
    in_maps = _host_inputs(**inputs)
    res = run_bass_kernel_spmd(nc, in_maps, core_ids=list(range(8)))
    outs = [res.results[c]["out"] for c in range(8)]
    user_output = np.concatenate(outs[0:4], axis=0)
    item_output = np.concatenate(outs[4:8], axis=0)
    return user_output, item_output
